# revision 17
# baseline (speedup 1.0000x reference)
"""AgentAttention Trainium2 kernel — 8-core batch-parallel (2 batches/core).

v2 restructure (validated in mirror.py):
  - agent tokens = pool(x) @ q_w computed on HOST (pooling is linear);
    stage-1 scores folded: s1 = x @ M1 with M1 = k_w @ (scale*agent)^T,
    stage-2 scores folded: s2 = x @ M2 with M2 = (scale*q_w) @ agent^T and
    the q_b term folded into the per-batch exp-bias eb2. This removes the
    device Q and K projections, the PSUM->SBUF q/k copies, and the device
    pooling reduces entirely.
  - position biases as exp() factors (multiplicative), eb1 persistent in
    SBUF (constant across batches), eb2 streamed per batch.
  - depthwise 3x3 conv split across engines (tunable): diag-matmul units on
    the PE, fused scalar_tensor_tensor chains on DVE and GpSimd.
  - stage-1 fused chunk loop: V projected just-in-time, ones-augmented V for
    the softmax denominator; stage-2 in s2^T (agent-partition) layout;
    denominators via ones-matmul, reciprocal broadcast via DRAM roundtrip.
"""
import numpy as np
import ml_dtypes

BF = ml_dtypes.bfloat16
NCORES = 8
B = 2              # batches per core
N = 3136
H = W = 56
CT = 4             # 128-channel tiles
HP = 4             # head pairs
A = 49
C7 = 448           # 8 image rows
CH = [(i * 128, min(128, N - i * 128)) for i in range(25)]

# dwc unit split: 28 (hp, c) units per batch across engines
DWC_DVE = 8        # units on Vector
DWC_GPS = 0        # units on GpSimd (Pool engine rejects STT in codegen)
# remaining 28 - DWC_DVE - DWC_GPS stay on the PE

_CACHE = {}


def _lin_weights(in_size, out_size):
    scale = in_size / out_size
    src = (np.arange(out_size, dtype=np.float32) + 0.5) * scale - 0.5
    src = np.maximum(src, 0.0)
    i0 = np.minimum(np.floor(src).astype(np.int32), in_size - 1)
    i1 = np.minimum(i0 + 1, in_size - 1)
    w = (src - i0.astype(np.float32)).astype(np.float32)
    return i0, i1, w


def _resize_matrix(in_size, out_size):
    i0, i1, w = _lin_weights(in_size, out_size)
    M = np.zeros((out_size, in_size), np.float32)
    M[np.arange(out_size), i0] += 1.0 - w
    M[np.arange(out_size), i1] += w
    return M


def _dwc_units():
    units = [(hp, c) for hp in range(HP) for c in range(7)]
    # spread DVE/GPS units across hp so vpad regions are touched evenly
    dve = units[0:DWC_DVE]
    gps = units[DWC_DVE:DWC_DVE + DWC_GPS]
    pe = units[DWC_DVE + DWC_GPS:]
    return pe, dve, gps


def _build_nc():
    from contextlib import ExitStack
    import concourse.bacc as bacc
    import concourse.tile as tile
    from concourse import mybir

    fp32 = mybir.dt.float32
    bf16 = mybir.dt.bfloat16
    AF = mybir.ActivationFunctionType
    OP = mybir.AluOpType
    AX = mybir.AxisListType

    PE_UNITS, DVE_UNITS, GPS_UNITS = _dwc_units()

    nc = bacc.Bacc("TRN2", target_bir_lowering=False)
    xT_d = nc.dram_tensor("xT", (128, B, CT, N), bf16, kind="ExternalInput")
    m1_d = nc.dram_tensor("m1", (128, B, CT, 512), bf16, kind="ExternalInput")
    m2_d = nc.dram_tensor("m2", (128, B, 16, 128), bf16, kind="ExternalInput")
    wv_d = nc.dram_tensor("wv", (128, CT, 512), bf16, kind="ExternalInput")
    pw_d = nc.dram_tensor("pw", (128, CT, 512), bf16, kind="ExternalInput")
    wdiag_d = nc.dram_tensor("wdiag", (128, 36, 128), bf16, kind="ExternalInput")
    wdvec_d = nc.dram_tensor("wdvec", (128, CT, 9), fp32, kind="ExternalInput")
    eb1_d = nc.dram_tensor("eb1", (128, 25, HP, 128), bf16, kind="ExternalInput")
    eb2_d = nc.dram_tensor("eb2", (128, B, HP, 7, C7), bf16, kind="ExternalInput")
    ones_d = nc.dram_tensor("onesbd", (128, 2), bf16, kind="ExternalInput")
    out_d = nc.dram_tensor("out", (B, N, 512), fp32, kind="ExternalOutput")
    rsc_d = nc.dram_tensor("rscratch", (B, 2, HP, N), bf16, kind="Internal")

    with ExitStack() as ctx:
        tc = ctx.enter_context(tile.TileContext(nc))
        consts = ctx.enter_context(tc.tile_pool(name="consts", bufs=1))
        xu = ctx.enter_context(tc.tile_pool(name="xu", bufs=1))
        usp = ctx.enter_context(tc.tile_pool(name="usp", bufs=1))
        vdp = ctx.enter_context(tc.tile_pool(name="vdp", bufs=1))
        dwp = ctx.enter_context(tc.tile_pool(name="dwp", bufs=1))
        mbp = ctx.enter_context(tc.tile_pool(name="mbp", bufs=1))
        ebp = ctx.enter_context(tc.tile_pool(name="ebp", bufs=2))
        work = ctx.enter_context(tc.tile_pool(name="work", bufs=2))
        accp = ctx.enter_context(tc.tile_pool(name="accp", bufs=2))
        perb = ctx.enter_context(tc.tile_pool(name="perb", bufs=2))
        rbcp = ctx.enter_context(tc.tile_pool(name="rbcp", bufs=2))
        otp = ctx.enter_context(tc.tile_pool(name="otp", bufs=2))
        ps_mm = ctx.enter_context(tc.tile_pool(name="psmm", bufs=3, space="PSUM"))
        ps_s2 = ctx.enter_context(tc.tile_pool(name="pss2", bufs=2, space="PSUM"))
        ps_av = ctx.enter_context(tc.tile_pool(name="psav", bufs=2, space="PSUM"))
        ps_sm = ctx.enter_context(tc.tile_pool(name="pssm", bufs=1, space="PSUM"))

        wv_s = consts.tile([128, CT, 512], bf16)
        nc.sync.dma_start(out=wv_s, in_=wv_d[:, :, :])
        pw_s = consts.tile([128, CT, 512], bf16)
        nc.sync.dma_start(out=pw_s, in_=pw_d[:, :, :])
        wdiag_s = consts.tile([128, 36, 128], bf16)
        nc.sync.dma_start(out=wdiag_s, in_=wdiag_d[:, :, :])
        wdvec_s = consts.tile([128, CT, 9], fp32)
        nc.sync.dma_start(out=wdvec_s, in_=wdvec_d[:, :, :])
        eb1_s = consts.tile([128, 25, HP, 128], bf16)
        nc.sync.dma_start(out=eb1_s, in_=eb1_d[:, :, :, :])
        onesbd = consts.tile([128, 2], bf16)
        nc.sync.dma_start(out=onesbd, in_=ones_d[:, :])

        def phase_a(b, S):
            """DMA loads for batch b."""
            xT = xu.tile([128, CT, N], bf16, tag="xu")
            for kt in range(CT):
                nc.sync.dma_start(out=xT[:, kt, :], in_=xT_d[:, b, kt, :])
            m1_s = mbp.tile([128, CT, 512], bf16, tag="m1")
            nc.sync.dma_start(out=m1_s, in_=m1_d[:, b, :, :])
            m2_s = mbp.tile([128, 16, 128], bf16, tag="m2")
            nc.sync.dma_start(out=m2_s, in_=m2_d[:, b, :, :])
            S.update(xT=xT, m1_s=m1_s, m2_s=m2_s)

        def phase_b(b, S):
            """Stage 1: per-chunk s1 scores, JIT V, agent_v accumulation."""
            xT, m1_s = S['xT'], S['m1_s']
            vpad = vdp.tile([128, CT, 58, 58], bf16, tag="vpad")
            nc.vector.memset(vpad, 0.0)
            avp0 = ps_av.tile([128, 260], fp32, tag="av")
            avp1 = ps_av.tile([128, 260], fp32, tag="av")
            avt = [(avp0, 0), (avp0, 130), (avp1, 0), (avp1, 130)]
            for ci, (t0, cs) in enumerate(CH):
                # s1^T scores: [cs, 512] = x_chunk^T @ M1
                ps1 = ps_mm.tile([128, 512], fp32, tag="mm")
                for kt in range(CT):
                    nc.tensor.matmul(
                        ps1[0:cs, :], xT[:, kt, t0:t0 + cs], m1_s[:, kt, :],
                        start=(kt == 0), stop=(kt == 3),
                    )
                et4 = work.tile([128, HP, 128], bf16, tag="e1")
                nc.scalar.activation(
                    out=et4[0:cs, :, :].rearrange("p h a -> p (h a)"),
                    in_=ps1[0:cs, :], func=AF.Exp)
                nc.vector.tensor_mul(
                    out=et4[0:cs, :, :], in0=et4[0:cs, :, :],
                    in1=eb1_s[0:cs, ci, :, :])
                # V chunk: [cs, 512] then scatter into vpad + ones-augmented v65
                psV = ps_mm.tile([128, 512], fp32, tag="mm")
                for kt in range(CT):
                    nc.tensor.matmul(
                        psV[0:cs, :], xT[:, kt, t0:t0 + cs], wv_s[:, kt, :],
                        start=(kt == 0), stop=(kt == 3),
                    )
                v65 = perb.tile([128, 8, 65], bf16, tag="v65")
                nc.vector.tensor_copy(
                    out=v65[0:cs, :, 0:64],
                    in_=psV[0:cs, :].rearrange("p (h d) -> p h d", h=8),
                )
                nc.vector.memset(v65[0:cs, :, 64:65], 1.0)
                for hp in range(HP):
                    avp, off = avt[hp]
                    nc.tensor.matmul(
                        avp[:, off:off + 130],
                        et4[0:cs, hp, :],
                        v65[0:cs, 2 * hp:2 * hp + 2, :],
                        start=(ci == 0), stop=(ci == 24),
                    )
            S.update(vpad=vpad, avt=avt)

        def fill_vpad(b, S):
            """Scatter V rows into the padded image (from chunk-major v65 is
            not possible -- recompute V rows into vpad via tensor engine)."""
            # vpad rows are filled from a separate ch-major V pass
            xT, vpad = S['xT'], S['vpad']
            for ct in range(CT):
                for c in range(7):
                    ps = ps_mm.tile([128, 512], fp32, tag="mm")
                    for kt in range(CT):
                        nc.tensor.matmul(
                            ps[:, 0:C7],
                            wv_s[:, kt, ct * 128:(ct + 1) * 128],
                            xT[:, kt, c * C7:(c + 1) * C7],
                            start=(kt == 0), stop=(kt == 3),
                        )
                    nc.vector.tensor_copy(
                        out=vpad[:, ct, 1 + 8 * c:9 + 8 * c, 1:57],
                        in_=ps[:, 0:C7].rearrange("p (y x) -> p y x", y=8))

        def phase_av(b, S):
            """Extract normalized agent_v (block-diagonal head-pair lhsT)."""
            avt = S['avt']
            avbds = []
            for hp in range(HP):
                avp, off = avt[hp]
                avbd = perb.tile([128, 128], bf16, tag=f"avbd{hp}")
                nc.vector.memset(avbd, 0.0)
                rr = work.tile([128, 1], fp32, tag="rr")
                for e in range(2):
                    nc.vector.reciprocal(
                        out=rr[64 * e:64 * e + 49, :],
                        in_=avp[64 * e:64 * e + 49, off + 65 * e + 64:off + 65 * e + 65])
                    nc.vector.tensor_scalar_mul(
                        out=avbd[64 * e:64 * e + 49, 64 * e:64 * e + 64],
                        in0=avp[64 * e:64 * e + 49, off + 65 * e:off + 65 * e + 64],
                        scalar1=rr[64 * e:64 * e + 49, :],
                    )
                avbds.append(avbd)
            S.update(avbds=avbds)

        def phase_d(b, S):
            """Stage 2 + dwc: per (hp, c-group) s2 scores, psU/psD, dwc units."""
            xT, m2_s = S['xT'], S['m2_s']
            vpad, avbds = S['vpad'], S['avbds']
            u_s = usp.tile([128, CT, N], bf16, tag="us")
            dwc_s = dwp.tile([128, CT, N], bf16, tag="dwc")
            denpk = perb.tile([112, 224], fp32, tag="denpk")

            def emit_dwc_pe(hp, c):
                psW = ps_mm.tile([128, 512], fp32, tag="mm")
                for j in range(9):
                    dy, dx = j // 3, j % 3
                    nc.tensor.matmul(
                        psW[:, 0:C7],
                        wdiag_s[:, hp * 9 + j, :],
                        vpad[:, hp, 8 * c + dy:8 * c + dy + 8, dx:dx + 56],
                        start=(j == 0), stop=(j == 8),
                    )
                nc.vector.tensor_copy(
                    out=dwc_s[:, hp, c * C7:(c + 1) * C7], in_=psW[:, 0:C7])

            def emit_dwc_stt(eng, hp, c):
                accA = work.tile([128, C7], bf16, tag="dacc")
                accB = work.tile([128, C7], bf16, tag="dacc")
                dst = dwc_s[:, hp, c * C7:(c + 1) * C7].rearrange(
                    "p (y x) -> p y x", y=8)
                accs = [accA.rearrange("p (y x) -> p y x", y=8),
                        accB.rearrange("p (y x) -> p y x", y=8)]
                eng.tensor_scalar_mul(
                    out=accs[0],
                    in0=vpad[:, hp, 8 * c:8 * c + 8, 0:56],
                    scalar1=wdvec_s[:, hp, 0:1],
                )
                for j in range(1, 9):
                    dy, dx = j // 3, j % 3
                    o = dst if j == 8 else accs[j % 2]
                    eng.scalar_tensor_tensor(
                        out=o,
                        in0=vpad[:, hp, 8 * c + dy:8 * c + dy + 8, dx:dx + 56],
                        scalar=wdvec_s[:, hp, j:j + 1],
                        in1=accs[(j - 1) % 2],
                        op0=OP.mult, op1=OP.add,
                    )

            dve_q = list(DVE_UNITS)
            gps_q = list(GPS_UNITS)
            pe_q = list(PE_UNITS)
            for hp in range(HP):
                eb2_s = ebp.tile([128, 7, C7], bf16, tag="eb2")
                nc.gpsimd.dma_start(out=eb2_s, in_=eb2_d[:, b, hp, :, :])
                for cg in ((0, 1), (2, 3), (4, 5), (6,)):
                    ps2t = {}
                    for c in cg:
                        ps2t[c] = ps_s2.tile([128, 512], fp32, tag="s2",
                                             name=f"ps2_{hp}_{c}")
                    for kt in range(CT):
                        for c in cg:
                            nc.tensor.matmul(
                                ps2t[c][0:128, 0:C7],
                                m2_s[:, kt * 4 + hp, :],
                                xT[:, kt, c * C7:(c + 1) * C7],
                                start=(kt == 0), stop=(kt == 3),
                            )
                    for c in cg:
                        et2 = work.tile([128, C7], bf16, tag="e2")
                        nc.scalar.activation(out=et2, in_=ps2t[c][0:128, 0:C7],
                                             func=AF.Exp)
                        nc.vector.tensor_mul(out=et2, in0=et2,
                                             in1=eb2_s[:, c, :])
                        psU = ps_mm.tile([128, 512], fp32, tag="mm")
                        nc.tensor.matmul(psU[:, 0:C7], avbds[hp], et2,
                                         start=True, stop=True)
                        psD = ps_sm.tile([2, C7], fp32, tag="sm")
                        nc.tensor.matmul(psD, onesbd, et2, start=True, stop=True)
                        nc.scalar.copy(out=u_s[:, hp, c * C7:(c + 1) * C7],
                                       in_=psU[:, 0:C7])
                        dtmp = work.tile([2, C7], fp32, tag="dtmp")
                        nc.scalar.copy(out=dtmp, in_=psD)
                        for e in range(2):
                            nc.gpsimd.dma_start(
                                out=denpk[e * 56 + hp * 14 + 2 * c:
                                          e * 56 + hp * 14 + 2 * c + 2, :],
                                in_=dtmp[e:e + 1, :])
                        # interleave dwc units to overlap engines
                        if dve_q:
                            emit_dwc_stt(nc.vector, *dve_q.pop(0))
                        if gps_q:
                            emit_dwc_stt(nc.gpsimd, *gps_q.pop(0))
                        if pe_q:
                            emit_dwc_pe(*pe_q.pop(0))
            for u_ in pe_q:
                emit_dwc_pe(*u_)
            for u_ in dve_q:
                emit_dwc_stt(nc.vector, *u_)
            for u_ in gps_q:
                emit_dwc_stt(nc.gpsimd, *u_)
            rpk = perb.tile([112, 224], bf16, tag="rpk")
            with nc.allow_low_precision(reason="single bf16 rounding of 1/den"):
                nc.vector.reciprocal(out=rpk, in_=denpk)
            nc.sync.dma_start(out=rsc_d[b, :, :, :], in_=rpk)
            S.update(u_s=u_s, dwc_s=dwc_s)

        def phase_c_chunk(b, S, c):
            u_s, dwc_s = S['u_s'], S['dwc_s']
            sl = slice(c * C7, (c + 1) * C7)
            rbc4 = rbcp.tile([128, CT, C7], bf16, tag="rbc")
            for e in range(2):
                nc.gpsimd.dma_start(
                    out=rbc4[64 * e:64 * e + 64, :, :],
                    in_=rsc_d[b, e:e + 1, :, c * C7:(c + 1) * C7].to_broadcast(
                        (64, CT, C7)))
            for ct in range(CT):
                nc.vector.tensor_mul(out=u_s[:, ct, sl], in0=u_s[:, ct, sl],
                                     in1=rbc4[:, ct, :])
                nc.vector.tensor_add(out=u_s[:, ct, sl], in0=u_s[:, ct, sl],
                                     in1=dwc_s[:, ct, sl])

        def phase_p_chunk(b, S, ci):
            u_s = S['u_s']
            t0, cs = CH[ci]
            psP = ps_mm.tile([128, 512], fp32, tag="mm")
            for kt in range(CT):
                nc.tensor.matmul(
                    psP[0:cs, :], u_s[:, kt, t0:t0 + cs], pw_s[:, kt, :],
                    start=(kt == 0), stop=(kt == 3),
                )
            ot = otp.tile([128, 512], fp32, tag="ot")
            nc.scalar.copy(out=ot[0:cs, :], in_=psP[0:cs, :])
            nc.sync.dma_start(out=out_d[b, t0:t0 + cs, :], in_=ot[0:cs, :])

        def phase_cp(b, S):
            """Interleave normalization+add with projection, chunk by chunk."""
            done = 0
            for c in range(7):
                phase_c_chunk(b, S, c)
                lim = (c + 1) * C7
                while done < 25 and CH[done][0] + CH[done][1] <= lim:
                    phase_p_chunk(b, S, done)
                    done += 1
            while done < 25:
                phase_p_chunk(b, S, done)
                done += 1

        S0, S1 = {}, {}
        phase_a(0, S0)
        phase_b(0, S0)
        fill_vpad(0, S0)
        phase_av(0, S0)
        phase_d(0, S0)
        phase_a(1, S1)
        phase_cp(0, S0)
        phase_b(1, S1)
        fill_vpad(1, S1)
        phase_av(1, S1)
        phase_d(1, S1)
        phase_cp(1, S1)
    return nc


def _host_prep(x, q_w, q_b, kv_w, kv_b, proj_w, proj_b, dwc_w, dwc_b,
               an_bias, na_bias, ah_bias, aw_bias, ha_bias, wa_bias):
    heads, dh = 8, 64
    b = x.shape[0]
    ID = 512
    scale = dh ** -0.5
    q_w = np.asarray(q_w, np.float32); q_b = np.asarray(q_b, np.float32)
    kv_w = np.asarray(kv_w, np.float32); kv_b = np.asarray(kv_b, np.float32)
    proj_w = np.asarray(proj_w, np.float32); proj_b = np.asarray(proj_b, np.float32)
    dwc_w = np.asarray(dwc_w, np.float32); dwc_b = np.asarray(dwc_b, np.float32)

    Rh = _resize_matrix(7, H)
    Rw = _resize_matrix(7, W)
    an = np.asarray(an_bias, np.float32); na = np.asarray(na_bias, np.float32)
    pb1 = np.einsum('yi,haij,xj->hayx', Rh, an, Rw).reshape(heads, A, N)
    pb2 = (np.asarray(ah_bias, np.float32) + np.asarray(aw_bias, np.float32)).reshape(heads, A, N)
    bias1 = pb1 + pb2                                      # (h, a, n)
    ab1 = np.einsum('yi,haij,xj->hayx', Rh, na, Rw).reshape(heads, A, N)
    ab2 = (np.asarray(ha_bias, np.float32) + np.asarray(wa_bias, np.float32)).reshape(heads, N, A)
    bias2 = ab1.transpose(0, 2, 1) + ab2                   # (h, n, a)

    k_w = kv_w[:, :ID]
    v_w = kv_w[:, ID:]
    v_b = kv_b[ID:]
    dwc9 = dwc_w.reshape(ID, 9)

    # host agent tokens + folded score matrices
    xi = x.reshape(b, 7, 8, 7, 8, ID)
    px = xi.mean(axis=(2, 4)).reshape(b, A, ID)
    agent = px @ q_w + q_b[None, None, :]                  # (b, 49, 512)
    agent_h = agent.reshape(b, A, heads, dh).transpose(0, 2, 1, 3)
    k_wh = k_w.reshape(ID, heads, dh)
    q_wh = q_w.reshape(ID, heads, dh)
    M1 = np.einsum('chd,bhad->bcha', k_wh, agent_h * scale)   # (b, 512, h, 49)
    M2 = np.einsum('chd,bhad->bcha', q_wh, agent_h * scale)
    qbag = np.einsum('hd,bhad->bha', (q_b * scale).reshape(heads, dh), agent_h)

    # m1 packed (128, B(global), CT, 512): rhs for s1; col hp*128 + 64e + a
    m1p = np.zeros((b, 512, CT, 128), np.float32)          # (b, ch, hp-slot, col)
    m1c = np.zeros((b, 128, CT, 512), np.float32)
    m2c = np.zeros((b, 128, 16, 128), np.float32)
    for hp_ in range(HP):
        for e in range(2):
            col = slice(64 * e, 64 * e + 49)
            m1p[:, :, hp_, col] = M1[:, :, 2 * hp_ + e, :]
    m1r = m1p.reshape(b, CT, 128, CT * 128)                # ch=(kt,p) -> [b,kt,p,512]
    m1c = np.ascontiguousarray(m1r.transpose(0, 2, 1, 3))  # (b, 128, CT, 512)
    for kt in range(CT):
        for hp_ in range(HP):
            for e in range(2):
                m2c[:, :, kt * 4 + hp_, 64 * e:64 * e + 49] = \
                    M2[:, kt * 128:(kt + 1) * 128, 2 * hp_ + e, :]
    m1_t = m1c.astype(BF)
    m2_t = np.ascontiguousarray(m2c).astype(BF)

    wv_t = np.ascontiguousarray(v_w.reshape(4, 128, 512).transpose(1, 0, 2)).astype(BF)
    pw_t = np.ascontiguousarray(proj_w.reshape(4, 128, 512).transpose(1, 0, 2)).astype(BF)
    wdiag_t = np.zeros((128, 36, 128), np.float32)
    for ct_ in range(4):
        for j_ in range(9):
            wdiag_t[np.arange(128), ct_ * 9 + j_, np.arange(128)] = \
                dwc9[ct_ * 128 + np.arange(128), j_]
    wdiag_t = wdiag_t.astype(BF)
    wdvec_t = np.ascontiguousarray(
        dwc9.reshape(4, 128, 9).transpose(1, 0, 2)).astype(np.float32)

    # eb1 (128, 25, HP, 128): [p, ci, hp, 64e+a] = exp(bias1)[2hp+e, a, 128ci+p]
    e1 = np.exp(bias1)                                     # (h, a, n)
    e1p = np.ones((128, 25, HP, 128), np.float32)
    e1t = e1.transpose(2, 0, 1)                            # (n, h, a)
    for ci, (t0, cs) in enumerate(CH):
        blk = e1t[t0:t0 + cs]
        for hp_ in range(HP):
            e1p[:cs, ci, hp_, 0:49] = blk[:, 2 * hp_, :]
            e1p[:cs, ci, hp_, 64:113] = blk[:, 2 * hp_ + 1, :]
    eb1_t = e1p.astype(BF)

    # eb2 (128, b, HP, 7, 448): [64e+a, bi, hp, c, t'] =
    #   exp(bias2)[2hp+e, 448c+t', a] * exp(qbag)[bi, 2hp+e, a]
    e2 = np.exp(bias2)                                     # (h, n, a)
    eqb = np.exp(qbag)                                     # (b, h, a)
    e2p = np.zeros((128, b, HP, 7, C7), np.float32)
    for hp_ in range(HP):
        for e in range(2):
            base = e2[2 * hp_ + e].reshape(7, C7, A).transpose(2, 0, 1)  # (A,7,C7)
            for bi in range(b):
                e2p[64 * e:64 * e + 49, bi, hp_] = \
                    base * eqb[bi, 2 * hp_ + e][:, None, None]
    eb2_t = e2p.astype(BF)

    ones_t = np.zeros((128, 2), np.float32)
    ones_t[0:49, 0] = 1.0
    ones_t[64:113, 1] = 1.0
    ones_t = ones_t.astype(BF)

    # host additive correction (v_b + dwc_b + proj_b, exact via softmax-sum-1)
    Mv = np.zeros((9, H, W), np.float32)
    for j in range(9):
        dy, dx = j // 3 - 1, j % 3 - 1
        Mv[j, max(0, -dy):H - max(0, dy), max(0, -dx):W - max(0, dx)] = 1.0
    Smat = np.einsum('jt,cj->tc', Mv.reshape(9, N), dwc9)
    corr = v_b[None, :] * (1.0 + Smat) + dwc_b[None, :]
    corr_out = (corr @ proj_w + proj_b[None, :]).astype(np.float32)

    shared = dict(wv=wv_t, pw=pw_t, wdiag=wdiag_t, wdvec=wdvec_t,
                  eb1=eb1_t, onesbd=ones_t)
    return shared, m1_t, m2_t, eb2_t, corr_out


def kernel(**inputs):
    from concourse.bass_utils import run_bass_kernel_spmd

    x = np.asarray(inputs['x'], np.float32)                # (16, 3136, 512)
    shared, m1_t, m2_t, eb2_t, corr_out = _host_prep(
        x, inputs['q_w'], inputs['q_b'], inputs['kv_w'], inputs['kv_b'],
        inputs['proj_w'], inputs['proj_b'], inputs['dwc_w'], inputs['dwc_b'],
        inputs['an_bias'], inputs['na_bias'], inputs['ah_bias'],
        inputs['aw_bias'], inputs['ha_bias'], inputs['wa_bias'])

    # xT per core: (128, B, CT, N) bf16 ; [p, b, kt, t] = x[2c+b, t, 128kt+p]
    xb = np.ascontiguousarray(
        x.reshape(NCORES, B, N, CT, 128).transpose(0, 4, 1, 3, 2)).astype(BF)
    # per-core batch-dependent tensors
    m1b = np.ascontiguousarray(
        m1_t.reshape(NCORES, B, 128, CT, 512).transpose(0, 2, 1, 3, 4))
    m2b = np.ascontiguousarray(
        m2_t.reshape(NCORES, B, 128, 16, 128).transpose(0, 2, 1, 3, 4))
    eb2b = np.ascontiguousarray(
        eb2_t.reshape(128, NCORES, B, HP, 7, C7).transpose(1, 0, 2, 3, 4, 5))

    if 'nc' not in _CACHE:
        nc = _build_nc()
        nc.finalize()
        _CACHE['nc'] = nc
    nc = _CACHE['nc']

    in_maps = []
    for c in range(NCORES):
        m = {'xT': xb[c], 'm1': m1b[c], 'm2': m2b[c], 'eb2': eb2b[c]}
        m.update(shared)
        in_maps.append(m)
    res = run_bass_kernel_spmd(nc, in_maps, core_ids=list(range(NCORES)))
    outs = res.results
    full = np.concatenate([np.asarray(o['out']).reshape(B, N, 512) for o in outs], axis=0)
    full = full + corr_out[None, :, :]
    return full.astype(np.float32)


# revision 18
# speedup vs baseline: 1.0059x; 1.0059x over previous
"""AgentAttention Trainium2 kernel — 8-core batch-parallel (2 batches/core).

v2 restructure (validated in mirror.py):
  - agent tokens = pool(x) @ q_w computed on HOST (pooling is linear);
    stage-1 scores folded: s1 = x @ M1 with M1 = k_w @ (scale*agent)^T,
    stage-2 scores folded: s2 = x @ M2 with M2 = (scale*q_w) @ agent^T and
    the q_b term folded into the per-batch exp-bias eb2. This removes the
    device Q and K projections, the PSUM->SBUF q/k copies, and the device
    pooling reduces entirely.
  - position biases as exp() factors (multiplicative), eb1 persistent in
    SBUF (constant across batches), eb2 streamed per batch.
  - depthwise 3x3 conv split across engines (tunable): diag-matmul units on
    the PE, fused scalar_tensor_tensor chains on DVE and GpSimd.
  - stage-1 fused chunk loop: V projected just-in-time, ones-augmented V for
    the softmax denominator; stage-2 in s2^T (agent-partition) layout;
    denominators via ones-matmul, reciprocal broadcast via DRAM roundtrip.
"""
import numpy as np
import ml_dtypes

BF = ml_dtypes.bfloat16
NCORES = 8
B = 2              # batches per core
N = 3136
H = W = 56
CT = 4             # 128-channel tiles
HP = 4             # head pairs
A = 49
C7 = 448           # 8 image rows
CH = [(i * 128, min(128, N - i * 128)) for i in range(25)]

# dwc unit split: 28 (hp, c) units per batch across engines
DWC_DVE = 0        # units on Vector
DWC_GPS = 0        # units on GpSimd (Pool engine rejects STT in codegen)
# remaining 28 - DWC_DVE - DWC_GPS stay on the PE

_CACHE = {}


def _lin_weights(in_size, out_size):
    scale = in_size / out_size
    src = (np.arange(out_size, dtype=np.float32) + 0.5) * scale - 0.5
    src = np.maximum(src, 0.0)
    i0 = np.minimum(np.floor(src).astype(np.int32), in_size - 1)
    i1 = np.minimum(i0 + 1, in_size - 1)
    w = (src - i0.astype(np.float32)).astype(np.float32)
    return i0, i1, w


def _resize_matrix(in_size, out_size):
    i0, i1, w = _lin_weights(in_size, out_size)
    M = np.zeros((out_size, in_size), np.float32)
    M[np.arange(out_size), i0] += 1.0 - w
    M[np.arange(out_size), i1] += w
    return M


def _dwc_units():
    units = [(hp, c) for hp in range(HP) for c in range(7)]
    # spread DVE/GPS units across hp so vpad regions are touched evenly
    dve = units[0:DWC_DVE]
    gps = units[DWC_DVE:DWC_DVE + DWC_GPS]
    pe = units[DWC_DVE + DWC_GPS:]
    return pe, dve, gps


def _build_nc():
    from contextlib import ExitStack
    import concourse.bacc as bacc
    import concourse.tile as tile
    from concourse import mybir

    fp32 = mybir.dt.float32
    bf16 = mybir.dt.bfloat16
    AF = mybir.ActivationFunctionType
    OP = mybir.AluOpType
    AX = mybir.AxisListType

    PE_UNITS, DVE_UNITS, GPS_UNITS = _dwc_units()

    nc = bacc.Bacc("TRN2", target_bir_lowering=False)
    xT_d = nc.dram_tensor("xT", (128, B, CT, N), bf16, kind="ExternalInput")
    m1_d = nc.dram_tensor("m1", (128, B, CT, 512), bf16, kind="ExternalInput")
    m2_d = nc.dram_tensor("m2", (128, B, 16, 128), bf16, kind="ExternalInput")
    wv_d = nc.dram_tensor("wv", (128, CT, 512), bf16, kind="ExternalInput")
    pw_d = nc.dram_tensor("pw", (128, CT, 512), bf16, kind="ExternalInput")
    wdiag_d = nc.dram_tensor("wdiag", (128, 36, 128), bf16, kind="ExternalInput")
    wdvec_d = nc.dram_tensor("wdvec", (128, CT, 9), fp32, kind="ExternalInput")
    eb1_d = nc.dram_tensor("eb1", (128, 25, HP, 128), bf16, kind="ExternalInput")
    eb2_d = nc.dram_tensor("eb2", (128, B, HP, 7, C7), bf16, kind="ExternalInput")
    ones_d = nc.dram_tensor("onesbd", (128, 2), bf16, kind="ExternalInput")
    out_d = nc.dram_tensor("out", (B, N, 512), fp32, kind="ExternalOutput")
    rsc_d = nc.dram_tensor("rscratch", (B, 2, HP, N), bf16, kind="Internal")

    with ExitStack() as ctx:
        tc = ctx.enter_context(tile.TileContext(nc))
        consts = ctx.enter_context(tc.tile_pool(name="consts", bufs=1))
        xu = ctx.enter_context(tc.tile_pool(name="xu", bufs=1))
        usp = ctx.enter_context(tc.tile_pool(name="usp", bufs=1))
        vdp = ctx.enter_context(tc.tile_pool(name="vdp", bufs=1))
        dwp = ctx.enter_context(tc.tile_pool(name="dwp", bufs=1))
        mbp = ctx.enter_context(tc.tile_pool(name="mbp", bufs=1))
        ebp = ctx.enter_context(tc.tile_pool(name="ebp", bufs=2))
        work = ctx.enter_context(tc.tile_pool(name="work", bufs=2))
        accp = ctx.enter_context(tc.tile_pool(name="accp", bufs=2))
        perb = ctx.enter_context(tc.tile_pool(name="perb", bufs=2))
        rbcp = ctx.enter_context(tc.tile_pool(name="rbcp", bufs=2))
        otp = ctx.enter_context(tc.tile_pool(name="otp", bufs=2))
        ps_mm = ctx.enter_context(tc.tile_pool(name="psmm", bufs=3, space="PSUM"))
        ps_s2 = ctx.enter_context(tc.tile_pool(name="pss2", bufs=2, space="PSUM"))
        ps_av = ctx.enter_context(tc.tile_pool(name="psav", bufs=2, space="PSUM"))
        ps_sm = ctx.enter_context(tc.tile_pool(name="pssm", bufs=1, space="PSUM"))

        wv_s = consts.tile([128, CT, 512], bf16)
        nc.sync.dma_start(out=wv_s, in_=wv_d[:, :, :])
        pw_s = consts.tile([128, CT, 512], bf16)
        nc.sync.dma_start(out=pw_s, in_=pw_d[:, :, :])
        wdiag_s = consts.tile([128, 36, 128], bf16)
        nc.sync.dma_start(out=wdiag_s, in_=wdiag_d[:, :, :])
        wdvec_s = consts.tile([128, CT, 9], fp32)
        nc.sync.dma_start(out=wdvec_s, in_=wdvec_d[:, :, :])
        eb1_s = consts.tile([128, 25, HP, 128], bf16)
        nc.sync.dma_start(out=eb1_s, in_=eb1_d[:, :, :, :])
        onesbd = consts.tile([128, 2], bf16)
        nc.sync.dma_start(out=onesbd, in_=ones_d[:, :])

        def phase_a(b, S):
            """DMA loads for batch b."""
            xT = xu.tile([128, CT, N], bf16, tag="xu")
            for kt in range(CT):
                nc.sync.dma_start(out=xT[:, kt, :], in_=xT_d[:, b, kt, :])
            m1_s = mbp.tile([128, CT, 512], bf16, tag="m1")
            nc.sync.dma_start(out=m1_s, in_=m1_d[:, b, :, :])
            m2_s = mbp.tile([128, 16, 128], bf16, tag="m2")
            nc.sync.dma_start(out=m2_s, in_=m2_d[:, b, :, :])
            S.update(xT=xT, m1_s=m1_s, m2_s=m2_s)

        def phase_b(b, S):
            """Stage 1: per-chunk s1 scores, JIT V, agent_v accumulation."""
            xT, m1_s = S['xT'], S['m1_s']
            vpad = vdp.tile([128, CT, 58, 58], bf16, tag="vpad")
            nc.vector.memset(vpad, 0.0)
            avp0 = ps_av.tile([128, 260], fp32, tag="av")
            avp1 = ps_av.tile([128, 260], fp32, tag="av")
            avt = [(avp0, 0), (avp0, 130), (avp1, 0), (avp1, 130)]
            for ci, (t0, cs) in enumerate(CH):
                # s1^T scores: [cs, 512] = x_chunk^T @ M1
                ps1 = ps_mm.tile([128, 512], fp32, tag="mm")
                for kt in range(CT):
                    nc.tensor.matmul(
                        ps1[0:cs, :], xT[:, kt, t0:t0 + cs], m1_s[:, kt, :],
                        start=(kt == 0), stop=(kt == 3),
                    )
                et4 = work.tile([128, HP, 128], bf16, tag="e1")
                nc.scalar.activation(
                    out=et4[0:cs, :, :].rearrange("p h a -> p (h a)"),
                    in_=ps1[0:cs, :], func=AF.Exp)
                nc.vector.tensor_mul(
                    out=et4[0:cs, :, :], in0=et4[0:cs, :, :],
                    in1=eb1_s[0:cs, ci, :, :])
                # V chunk: [cs, 512] then scatter into vpad + ones-augmented v65
                psV = ps_mm.tile([128, 512], fp32, tag="mm")
                for kt in range(CT):
                    nc.tensor.matmul(
                        psV[0:cs, :], xT[:, kt, t0:t0 + cs], wv_s[:, kt, :],
                        start=(kt == 0), stop=(kt == 3),
                    )
                v65 = perb.tile([128, 8, 65], bf16, tag="v65")
                nc.vector.tensor_copy(
                    out=v65[0:cs, :, 0:64],
                    in_=psV[0:cs, :].rearrange("p (h d) -> p h d", h=8),
                )
                nc.vector.memset(v65[0:cs, :, 64:65], 1.0)
                for hp in range(HP):
                    avp, off = avt[hp]
                    nc.tensor.matmul(
                        avp[:, off:off + 130],
                        et4[0:cs, hp, :],
                        v65[0:cs, 2 * hp:2 * hp + 2, :],
                        start=(ci == 0), stop=(ci == 24),
                    )
            S.update(vpad=vpad, avt=avt)

        def fill_vpad(b, S):
            """Scatter V rows into the padded image (from chunk-major v65 is
            not possible -- recompute V rows into vpad via tensor engine)."""
            # vpad rows are filled from a separate ch-major V pass
            xT, vpad = S['xT'], S['vpad']
            for ct in range(CT):
                for c in range(7):
                    ps = ps_mm.tile([128, 512], fp32, tag="mm")
                    for kt in range(CT):
                        nc.tensor.matmul(
                            ps[:, 0:C7],
                            wv_s[:, kt, ct * 128:(ct + 1) * 128],
                            xT[:, kt, c * C7:(c + 1) * C7],
                            start=(kt == 0), stop=(kt == 3),
                        )
                    nc.vector.tensor_copy(
                        out=vpad[:, ct, 1 + 8 * c:9 + 8 * c, 1:57],
                        in_=ps[:, 0:C7].rearrange("p (y x) -> p y x", y=8))

        def phase_av(b, S):
            """Extract normalized agent_v (block-diagonal head-pair lhsT)."""
            avt = S['avt']
            avbds = []
            for hp in range(HP):
                avp, off = avt[hp]
                avbd = perb.tile([128, 128], bf16, tag=f"avbd{hp}")
                nc.vector.memset(avbd, 0.0)
                rr = work.tile([128, 1], fp32, tag="rr")
                for e in range(2):
                    nc.vector.reciprocal(
                        out=rr[64 * e:64 * e + 49, :],
                        in_=avp[64 * e:64 * e + 49, off + 65 * e + 64:off + 65 * e + 65])
                    nc.vector.tensor_scalar_mul(
                        out=avbd[64 * e:64 * e + 49, 64 * e:64 * e + 64],
                        in0=avp[64 * e:64 * e + 49, off + 65 * e:off + 65 * e + 64],
                        scalar1=rr[64 * e:64 * e + 49, :],
                    )
                avbds.append(avbd)
            S.update(avbds=avbds)

        def phase_d(b, S):
            """Stage 2 + dwc: per (hp, c-group) s2 scores, psU/psD, dwc units."""
            xT, m2_s = S['xT'], S['m2_s']
            vpad, avbds = S['vpad'], S['avbds']
            u_s = usp.tile([128, CT, N], bf16, tag="us")
            dwc_s = dwp.tile([128, CT, N], bf16, tag="dwc")
            denpk = perb.tile([112, 224], fp32, tag="denpk")

            def emit_dwc_pe(hp, c):
                psW = ps_mm.tile([128, 512], fp32, tag="mm")
                for j in range(9):
                    dy, dx = j // 3, j % 3
                    nc.tensor.matmul(
                        psW[:, 0:C7],
                        wdiag_s[:, hp * 9 + j, :],
                        vpad[:, hp, 8 * c + dy:8 * c + dy + 8, dx:dx + 56],
                        start=(j == 0), stop=(j == 8),
                    )
                nc.vector.tensor_copy(
                    out=dwc_s[:, hp, c * C7:(c + 1) * C7], in_=psW[:, 0:C7])

            def emit_dwc_stt(eng, hp, c):
                accA = work.tile([128, C7], bf16, tag="dacc")
                accB = work.tile([128, C7], bf16, tag="dacc")
                dst = dwc_s[:, hp, c * C7:(c + 1) * C7].rearrange(
                    "p (y x) -> p y x", y=8)
                accs = [accA.rearrange("p (y x) -> p y x", y=8),
                        accB.rearrange("p (y x) -> p y x", y=8)]
                eng.tensor_scalar_mul(
                    out=accs[0],
                    in0=vpad[:, hp, 8 * c:8 * c + 8, 0:56],
                    scalar1=wdvec_s[:, hp, 0:1],
                )
                for j in range(1, 9):
                    dy, dx = j // 3, j % 3
                    o = dst if j == 8 else accs[j % 2]
                    eng.scalar_tensor_tensor(
                        out=o,
                        in0=vpad[:, hp, 8 * c + dy:8 * c + dy + 8, dx:dx + 56],
                        scalar=wdvec_s[:, hp, j:j + 1],
                        in1=accs[(j - 1) % 2],
                        op0=OP.mult, op1=OP.add,
                    )

            dve_q = list(DVE_UNITS)
            gps_q = list(GPS_UNITS)
            pe_q = list(PE_UNITS)
            for hp in range(HP):
                eb2_s = ebp.tile([128, 7, C7], bf16, tag="eb2")
                nc.gpsimd.dma_start(out=eb2_s, in_=eb2_d[:, b, hp, :, :])
                for cg in ((0, 1), (2, 3), (4, 5), (6,)):
                    ps2t = {}
                    for c in cg:
                        ps2t[c] = ps_s2.tile([128, 512], fp32, tag="s2",
                                             name=f"ps2_{hp}_{c}")
                    for kt in range(CT):
                        for c in cg:
                            nc.tensor.matmul(
                                ps2t[c][0:128, 0:C7],
                                m2_s[:, kt * 4 + hp, :],
                                xT[:, kt, c * C7:(c + 1) * C7],
                                start=(kt == 0), stop=(kt == 3),
                            )
                    for c in cg:
                        et2 = work.tile([128, C7], bf16, tag="e2")
                        nc.scalar.activation(out=et2, in_=ps2t[c][0:128, 0:C7],
                                             func=AF.Exp)
                        nc.vector.tensor_mul(out=et2, in0=et2,
                                             in1=eb2_s[:, c, :])
                        psU = ps_mm.tile([128, 512], fp32, tag="mm")
                        nc.tensor.matmul(psU[:, 0:C7], avbds[hp], et2,
                                         start=True, stop=True)
                        psD = ps_sm.tile([2, C7], fp32, tag="sm")
                        nc.tensor.matmul(psD, onesbd, et2, start=True, stop=True)
                        nc.scalar.copy(out=u_s[:, hp, c * C7:(c + 1) * C7],
                                       in_=psU[:, 0:C7])
                        dtmp = work.tile([2, C7], fp32, tag="dtmp")
                        nc.scalar.copy(out=dtmp, in_=psD)
                        for e in range(2):
                            nc.gpsimd.dma_start(
                                out=denpk[e * 56 + hp * 14 + 2 * c:
                                          e * 56 + hp * 14 + 2 * c + 2, :],
                                in_=dtmp[e:e + 1, :])
                        # interleave dwc units to overlap engines
                        if dve_q:
                            emit_dwc_stt(nc.vector, *dve_q.pop(0))
                        if gps_q:
                            emit_dwc_stt(nc.gpsimd, *gps_q.pop(0))
                        if pe_q:
                            emit_dwc_pe(*pe_q.pop(0))
            for u_ in pe_q:
                emit_dwc_pe(*u_)
            for u_ in dve_q:
                emit_dwc_stt(nc.vector, *u_)
            for u_ in gps_q:
                emit_dwc_stt(nc.gpsimd, *u_)
            rpk = perb.tile([112, 224], bf16, tag="rpk")
            with nc.allow_low_precision(reason="single bf16 rounding of 1/den"):
                nc.vector.reciprocal(out=rpk, in_=denpk)
            nc.sync.dma_start(out=rsc_d[b, :, :, :], in_=rpk)
            S.update(u_s=u_s, dwc_s=dwc_s)

        def phase_c_chunk(b, S, c):
            u_s, dwc_s = S['u_s'], S['dwc_s']
            sl = slice(c * C7, (c + 1) * C7)
            rbc4 = rbcp.tile([128, CT, C7], bf16, tag="rbc")
            for e in range(2):
                nc.gpsimd.dma_start(
                    out=rbc4[64 * e:64 * e + 64, :, :],
                    in_=rsc_d[b, e:e + 1, :, c * C7:(c + 1) * C7].to_broadcast(
                        (64, CT, C7)))
            for ct in range(CT):
                nc.vector.tensor_mul(out=u_s[:, ct, sl], in0=u_s[:, ct, sl],
                                     in1=rbc4[:, ct, :])
                nc.vector.tensor_add(out=u_s[:, ct, sl], in0=u_s[:, ct, sl],
                                     in1=dwc_s[:, ct, sl])

        def phase_p_chunk(b, S, ci):
            u_s = S['u_s']
            t0, cs = CH[ci]
            psP = ps_mm.tile([128, 512], fp32, tag="mm")
            for kt in range(CT):
                nc.tensor.matmul(
                    psP[0:cs, :], u_s[:, kt, t0:t0 + cs], pw_s[:, kt, :],
                    start=(kt == 0), stop=(kt == 3),
                )
            ot = otp.tile([128, 512], fp32, tag="ot")
            nc.scalar.copy(out=ot[0:cs, :], in_=psP[0:cs, :])
            nc.sync.dma_start(out=out_d[b, t0:t0 + cs, :], in_=ot[0:cs, :])

        def phase_cp(b, S):
            """Interleave normalization+add with projection, chunk by chunk."""
            done = 0
            for c in range(7):
                phase_c_chunk(b, S, c)
                lim = (c + 1) * C7
                while done < 25 and CH[done][0] + CH[done][1] <= lim:
                    phase_p_chunk(b, S, done)
                    done += 1
            while done < 25:
                phase_p_chunk(b, S, done)
                done += 1

        S0, S1 = {}, {}
        phase_a(0, S0)
        phase_b(0, S0)
        fill_vpad(0, S0)
        phase_av(0, S0)
        phase_d(0, S0)
        phase_a(1, S1)
        phase_cp(0, S0)
        phase_b(1, S1)
        fill_vpad(1, S1)
        phase_av(1, S1)
        phase_d(1, S1)
        phase_cp(1, S1)
    return nc


def _host_prep(x, q_w, q_b, kv_w, kv_b, proj_w, proj_b, dwc_w, dwc_b,
               an_bias, na_bias, ah_bias, aw_bias, ha_bias, wa_bias):
    heads, dh = 8, 64
    b = x.shape[0]
    ID = 512
    scale = dh ** -0.5
    q_w = np.asarray(q_w, np.float32); q_b = np.asarray(q_b, np.float32)
    kv_w = np.asarray(kv_w, np.float32); kv_b = np.asarray(kv_b, np.float32)
    proj_w = np.asarray(proj_w, np.float32); proj_b = np.asarray(proj_b, np.float32)
    dwc_w = np.asarray(dwc_w, np.float32); dwc_b = np.asarray(dwc_b, np.float32)

    Rh = _resize_matrix(7, H)
    Rw = _resize_matrix(7, W)
    an = np.asarray(an_bias, np.float32); na = np.asarray(na_bias, np.float32)
    pb1 = np.einsum('yi,haij,xj->hayx', Rh, an, Rw).reshape(heads, A, N)
    pb2 = (np.asarray(ah_bias, np.float32) + np.asarray(aw_bias, np.float32)).reshape(heads, A, N)
    bias1 = pb1 + pb2                                      # (h, a, n)
    ab1 = np.einsum('yi,haij,xj->hayx', Rh, na, Rw).reshape(heads, A, N)
    ab2 = (np.asarray(ha_bias, np.float32) + np.asarray(wa_bias, np.float32)).reshape(heads, N, A)
    bias2 = ab1.transpose(0, 2, 1) + ab2                   # (h, n, a)

    k_w = kv_w[:, :ID]
    v_w = kv_w[:, ID:]
    v_b = kv_b[ID:]
    dwc9 = dwc_w.reshape(ID, 9)

    # host agent tokens + folded score matrices
    xi = x.reshape(b, 7, 8, 7, 8, ID)
    px = xi.mean(axis=(2, 4)).reshape(b, A, ID)
    agent = px @ q_w + q_b[None, None, :]                  # (b, 49, 512)
    agent_h = agent.reshape(b, A, heads, dh).transpose(0, 2, 1, 3)
    k_wh = k_w.reshape(ID, heads, dh)
    q_wh = q_w.reshape(ID, heads, dh)
    M1 = np.einsum('chd,bhad->bcha', k_wh, agent_h * scale)   # (b, 512, h, 49)
    M2 = np.einsum('chd,bhad->bcha', q_wh, agent_h * scale)
    qbag = np.einsum('hd,bhad->bha', (q_b * scale).reshape(heads, dh), agent_h)

    # m1 packed (128, B(global), CT, 512): rhs for s1; col hp*128 + 64e + a
    m1p = np.zeros((b, 512, CT, 128), np.float32)          # (b, ch, hp-slot, col)
    m1c = np.zeros((b, 128, CT, 512), np.float32)
    m2c = np.zeros((b, 128, 16, 128), np.float32)
    for hp_ in range(HP):
        for e in range(2):
            col = slice(64 * e, 64 * e + 49)
            m1p[:, :, hp_, col] = M1[:, :, 2 * hp_ + e, :]
    m1r = m1p.reshape(b, CT, 128, CT * 128)                # ch=(kt,p) -> [b,kt,p,512]
    m1c = np.ascontiguousarray(m1r.transpose(0, 2, 1, 3))  # (b, 128, CT, 512)
    for kt in range(CT):
        for hp_ in range(HP):
            for e in range(2):
                m2c[:, :, kt * 4 + hp_, 64 * e:64 * e + 49] = \
                    M2[:, kt * 128:(kt + 1) * 128, 2 * hp_ + e, :]
    m1_t = m1c.astype(BF)
    m2_t = np.ascontiguousarray(m2c).astype(BF)

    wv_t = np.ascontiguousarray(v_w.reshape(4, 128, 512).transpose(1, 0, 2)).astype(BF)
    pw_t = np.ascontiguousarray(proj_w.reshape(4, 128, 512).transpose(1, 0, 2)).astype(BF)
    wdiag_t = np.zeros((128, 36, 128), np.float32)
    for ct_ in range(4):
        for j_ in range(9):
            wdiag_t[np.arange(128), ct_ * 9 + j_, np.arange(128)] = \
                dwc9[ct_ * 128 + np.arange(128), j_]
    wdiag_t = wdiag_t.astype(BF)
    wdvec_t = np.ascontiguousarray(
        dwc9.reshape(4, 128, 9).transpose(1, 0, 2)).astype(np.float32)

    # eb1 (128, 25, HP, 128): [p, ci, hp, 64e+a] = exp(bias1)[2hp+e, a, 128ci+p]
    e1 = np.exp(bias1)                                     # (h, a, n)
    e1p = np.ones((128, 25, HP, 128), np.float32)
    e1t = e1.transpose(2, 0, 1)                            # (n, h, a)
    for ci, (t0, cs) in enumerate(CH):
        blk = e1t[t0:t0 + cs]
        for hp_ in range(HP):
            e1p[:cs, ci, hp_, 0:49] = blk[:, 2 * hp_, :]
            e1p[:cs, ci, hp_, 64:113] = blk[:, 2 * hp_ + 1, :]
    eb1_t = e1p.astype(BF)

    # eb2 (128, b, HP, 7, 448): [64e+a, bi, hp, c, t'] =
    #   exp(bias2)[2hp+e, 448c+t', a] * exp(qbag)[bi, 2hp+e, a]
    e2 = np.exp(bias2)                                     # (h, n, a)
    eqb = np.exp(qbag)                                     # (b, h, a)
    e2p = np.zeros((128, b, HP, 7, C7), np.float32)
    for hp_ in range(HP):
        for e in range(2):
            base = e2[2 * hp_ + e].reshape(7, C7, A).transpose(2, 0, 1)  # (A,7,C7)
            for bi in range(b):
                e2p[64 * e:64 * e + 49, bi, hp_] = \
                    base * eqb[bi, 2 * hp_ + e][:, None, None]
    eb2_t = e2p.astype(BF)

    ones_t = np.zeros((128, 2), np.float32)
    ones_t[0:49, 0] = 1.0
    ones_t[64:113, 1] = 1.0
    ones_t = ones_t.astype(BF)

    # host additive correction (v_b + dwc_b + proj_b, exact via softmax-sum-1)
    Mv = np.zeros((9, H, W), np.float32)
    for j in range(9):
        dy, dx = j // 3 - 1, j % 3 - 1
        Mv[j, max(0, -dy):H - max(0, dy), max(0, -dx):W - max(0, dx)] = 1.0
    Smat = np.einsum('jt,cj->tc', Mv.reshape(9, N), dwc9)
    corr = v_b[None, :] * (1.0 + Smat) + dwc_b[None, :]
    corr_out = (corr @ proj_w + proj_b[None, :]).astype(np.float32)

    shared = dict(wv=wv_t, pw=pw_t, wdiag=wdiag_t, wdvec=wdvec_t,
                  eb1=eb1_t, onesbd=ones_t)
    return shared, m1_t, m2_t, eb2_t, corr_out


def kernel(**inputs):
    from concourse.bass_utils import run_bass_kernel_spmd

    x = np.asarray(inputs['x'], np.float32)                # (16, 3136, 512)
    shared, m1_t, m2_t, eb2_t, corr_out = _host_prep(
        x, inputs['q_w'], inputs['q_b'], inputs['kv_w'], inputs['kv_b'],
        inputs['proj_w'], inputs['proj_b'], inputs['dwc_w'], inputs['dwc_b'],
        inputs['an_bias'], inputs['na_bias'], inputs['ah_bias'],
        inputs['aw_bias'], inputs['ha_bias'], inputs['wa_bias'])

    # xT per core: (128, B, CT, N) bf16 ; [p, b, kt, t] = x[2c+b, t, 128kt+p]
    xb = np.ascontiguousarray(
        x.reshape(NCORES, B, N, CT, 128).transpose(0, 4, 1, 3, 2)).astype(BF)
    # per-core batch-dependent tensors
    m1b = np.ascontiguousarray(
        m1_t.reshape(NCORES, B, 128, CT, 512).transpose(0, 2, 1, 3, 4))
    m2b = np.ascontiguousarray(
        m2_t.reshape(NCORES, B, 128, 16, 128).transpose(0, 2, 1, 3, 4))
    eb2b = np.ascontiguousarray(
        eb2_t.reshape(128, NCORES, B, HP, 7, C7).transpose(1, 0, 2, 3, 4, 5))

    if 'nc' not in _CACHE:
        nc = _build_nc()
        nc.finalize()
        _CACHE['nc'] = nc
    nc = _CACHE['nc']

    in_maps = []
    for c in range(NCORES):
        m = {'xT': xb[c], 'm1': m1b[c], 'm2': m2b[c], 'eb2': eb2b[c]}
        m.update(shared)
        in_maps.append(m)
    res = run_bass_kernel_spmd(nc, in_maps, core_ids=list(range(NCORES)))
    outs = res.results
    full = np.concatenate([np.asarray(o['out']).reshape(B, N, 512) for o in outs], axis=0)
    full = full + corr_out[None, :, :]
    return full.astype(np.float32)
